# revision 8
# baseline (speedup 1.0000x reference)
"""BertSelfAttention (attention_type=0) on 8 Trainium2 NeuronCores.

Sharding: 32 (batch, head) pairs -> 4 pairs per core (same batch per core).
Each core computes QKV projections for its 4 heads on-chip (bf16 matmuls),
then attention with the additive spatial_score bias (fp32 softmax pipeline).

Per-core device pipeline:
  Phase 1 (QKV): per 128-row s-chunk, cast-DMA hidden/key/value to bf16,
    PE-transpose into [hid, s] layout, then matmuls against (pre-transposed,
    host-supplied) weight slices -> Q^T, K^T ([64,2048] per head, d-major)
    and V ([2048, 4*64] natural).
  Phase 2 (attention): per (head, q-chunk of 128):
    scores[128, 2048] = Q^T.T @ K^T into PSUM (bf16 MM, K=64), DVE-add the
    fp32 spatial tile, ACT exp -> bf16 probs U with per-row accumulated sums,
    PE block-transpose U -> U^T, PV matmuls (V stationary) -> ctx^T [64,128],
    transpose back, multiply by 1/rowsum, DMA out.

Host folds 1/sqrt(64) into Wq/bq, passes weight slices transposed, and adds
bv to the gathered output (exact: softmax rows sum to one).
"""

import json

import numpy as np

import concourse.bass as bass
import concourse.bass2jax as bass2jax
import concourse.mybir as mybir
import concourse.tile as tile
from concourse.bass_utils import compile_bir_kernel as _orig_compile_bir_kernel
from concourse.bass_utils import run_bass_kernel_spmd
from concourse.masks import make_identity

# ---------------------------------------------------------------------------
# The walrus build in this container accepts only ONE sync-wait per
# instruction ("Too many sync wait commands"), while Tile's semaphore
# assignment freely attaches several. Rewrite the BIR before walrus: move all
# but the last wait of each instruction onto injected same-engine NoOps
# placed directly before it (queue order preserved; waiting earlier on the
# same queue is always safe).
# ---------------------------------------------------------------------------


def _split_multi_waits(bir: bytes) -> bytes:
    j = json.loads(bir)
    for fn in j["functions"]:
        for blk in fn["blocks"]:
            new = []
            for inst in blk["instructions"]:
                si = inst.get("sync_info")
                waits = (si or {}).get("on_wait") or []
                if len(waits) > 1:
                    for idx, wv in enumerate(waits[:-1]):
                        new.append(
                            {
                                "debug": inst.get("debug", 0),
                                "engine": inst["engine"],
                                "ins": [],
                                "outs": [],
                                "name": f"{inst['name']}-w{idx}",
                                "opcode": "NoOp",
                                "sync_info": {"on_update": [], "on_wait": [wv]},
                            }
                        )
                    si["on_wait"] = [waits[-1]]
                new.append(inst)
            blk["instructions"] = new
    return json.dumps(j).encode()


def _patched_compile_bir_kernel(bir_json, tmpdir, neff_name="file.neff"):
    return _orig_compile_bir_kernel(_split_multi_waits(bir_json), tmpdir, neff_name)


bass2jax.compile_bir_kernel = _patched_compile_bir_kernel


def _install_ntff_hook():
    """Register the axon NTFF profile hook (absent antenv.axon_hooks shim)."""
    import sys
    import types

    if "antenv.axon_hooks" not in sys.modules:
        try:
            import antenv.axon_hooks  # noqa: F401
        except ImportError:
            import antenv

            mod = types.ModuleType("antenv.axon_hooks")
            holder = {}
            mod.set_axon_ntff_profile_hook = lambda h: holder.__setitem__("h", h)
            mod.get_axon_ntff_profile_hook = lambda: holder.get("h")
            sys.modules["antenv.axon_hooks"] = mod
            antenv.axon_hooks = mod
    from antenv.axon_hooks import (
        get_axon_ntff_profile_hook,
        set_axon_ntff_profile_hook,
    )

    if get_axon_ntff_profile_hook() is None:
        from trn_agent_boot.trn_boot import _ntff_profile_via_ctypes

        set_axon_ntff_profile_hook(
            _ntff_profile_via_ctypes("/opt/axon/libaxon_pjrt.so")
        )

B, S, H, DH = 2, 2048, 16, 64
HID = H * DH  # 1024
N_CORES = 8
HPC = (B * H) // N_CORES  # (b,h) pairs per core = 4, all same b
HJ = HID // 128  # 8 hid chunks
SC = S // 128  # 16 s-chunks
KW = 512  # k-slice width for scores
KT = S // KW  # 4 k tiles

_NC_CACHE = {}


def _build_kernel():
    f32 = mybir.dt.float32
    bf16 = mybir.dt.bfloat16

    nc = bass.Bass(trn_type="TRN2")
    hid_t = nc.dram_tensor("hid", (S, HID), f32, kind="ExternalInput")
    key_t = nc.dram_tensor("key", (S, HID), f32, kind="ExternalInput")
    val_t = nc.dram_tensor("val", (S, HID), f32, kind="ExternalInput")
    spat_t = nc.dram_tensor("spat", (HPC, S, S), f32, kind="ExternalInput")
    wq_t = nc.dram_tensor("wq_t", (HID, HPC * DH), f32, kind="ExternalInput")
    wk_t = nc.dram_tensor("wk_t", (HID, HPC * DH), f32, kind="ExternalInput")
    wv_t = nc.dram_tensor("wv_t", (HID, HPC * DH), f32, kind="ExternalInput")
    bq_t = nc.dram_tensor("bq2", (2, 128), f32, kind="ExternalInput")
    bk_t = nc.dram_tensor("bk2", (2, 128), f32, kind="ExternalInput")
    out_t = nc.dram_tensor("out", (S, HPC * DH), f32, kind="ExternalOutput")

    hid_ap, key_ap, val_ap = hid_t[:], key_t[:], val_t[:]
    spat_ap, out_ap = spat_t[:], out_t[:]

    with tile.TileContext(nc) as tc:
        with tc.tile_pool(name="singles", bufs=1) as singles:
            ident_bf = singles.tile([128, 128], bf16, tag="ident_bf")
            make_identity(nc, ident_bf)
            ident_f = singles.tile([128, 128], f32, tag="ident_f")
            make_identity(nc, ident_f)

            # weights, bf16, [128, HJ, 256] (hid-chunk-major)
            wq_sb = singles.tile([128, HJ, HPC * DH], bf16, tag="wq")
            wk_sb = singles.tile([128, HJ, HPC * DH], bf16, tag="wk")
            wv_sb = singles.tile([128, HJ, HPC * DH], bf16, tag="wv")
            nc.gpsimd.dma_start(
                out=wq_sb, in_=wq_t[:].rearrange("(c p) m -> p c m", p=128)
            )
            nc.gpsimd.dma_start(
                out=wk_sb, in_=wk_t[:].rearrange("(c p) m -> p c m", p=128)
            )
            nc.gpsimd.dma_start(
                out=wv_sb, in_=wv_t[:].rearrange("(c p) m -> p c m", p=128)
            )
            bq_sb = singles.tile([128, 2], f32, tag="bq")
            bk_sb = singles.tile([128, 2], f32, tag="bk")
            nc.gpsimd.dma_start(out=bq_sb, in_=bq_t[:].rearrange("h p -> p h"))
            nc.gpsimd.dma_start(out=bk_sb, in_=bk_t[:].rearrange("h p -> p h"))

            # persistent per-core QKV results
            qt_sb = [
                singles.tile([128, S], bf16, tag=f"qt{hp}", name=f"qt{hp}") for hp in range(2)
            ]  # [2 heads x 64d, s]
            kt_sb = [singles.tile([128, S], bf16, tag=f"kt{hp}", name=f"kt{hp}") for hp in range(2)]
            v_sb = singles.tile([128, SC, HPC * DH], bf16, tag="v")  # natural

            # ---------------- Phase 1: QKV projections ----------------
            with (
                tc.tile_pool(name="ph1", bufs=3) as ph1,
                tc.tile_pool(name="ph1t", bufs=3) as ph1t,
                tc.tile_pool(name="ph1_psT", bufs=2, space="PSUM") as ps_t1,
                tc.tile_pool(name="ph1_psQ", bufs=2, space="PSUM") as ps_q1,
            ):
                for i in range(SC):
                    rows = slice(i * 128, (i + 1) * 128)
                    h_nat = ph1.tile([128, HID], bf16, tag="h_nat")
                    k_nat = ph1.tile([128, HID], bf16, tag="k_nat")
                    v_nat = ph1.tile([128, HID], bf16, tag="v_nat")
                    nc.gpsimd.dma_start(out=h_nat, in_=hid_ap[rows, :])
                    nc.gpsimd.dma_start(out=k_nat, in_=key_ap[rows, :])
                    nc.gpsimd.dma_start(out=v_nat, in_=val_ap[rows, :])

                    hT = ph1t.tile([128, HJ, 128], bf16, tag="hT")
                    kT = ph1t.tile([128, HJ, 128], bf16, tag="kT")
                    vT = ph1t.tile([128, HJ, 128], bf16, tag="vT")
                    for src, dstT in ((h_nat, hT), (k_nat, kT), (v_nat, vT)):
                        for jj in range(HJ):
                            pt = ps_t1.tile([128, 128], f32, tag="pt")
                            nc.tensor.matmul(
                                pt,
                                lhsT=src[:, jj * 128 : (jj + 1) * 128],
                                rhs=ident_bf,
                                start=True,
                                stop=True,
                            )
                            if jj % 2 == 0:
                                nc.scalar.copy(dstT[:, jj, :], pt)
                            else:
                                nc.vector.tensor_copy(dstT[:, jj, :], pt)

                    # Q^T / K^T for this s-chunk (2 head-pairs each)
                    for hp in range(2):
                        cols = slice(hp * 128, (hp + 1) * 128)
                        qp = ps_q1.tile([128, 128], f32, tag="qp")
                        for jj in range(HJ):
                            nc.tensor.matmul(
                                qp,
                                lhsT=wq_sb[:, jj, cols],
                                rhs=hT[:, jj, :],
                                start=(jj == 0),
                                stop=(jj == HJ - 1),
                            )
                        nc.vector.tensor_scalar_add(
                            qt_sb[hp][:, rows], in0=qp, scalar1=bq_sb[:, hp : hp + 1]
                        )
                        kp = ps_q1.tile([128, 128], f32, tag="qp")
                        for jj in range(HJ):
                            nc.tensor.matmul(
                                kp,
                                lhsT=wk_sb[:, jj, cols],
                                rhs=kT[:, jj, :],
                                start=(jj == 0),
                                stop=(jj == HJ - 1),
                            )
                        nc.vector.tensor_scalar_add(
                            kt_sb[hp][:, rows], in0=kp, scalar1=bk_sb[:, hp : hp + 1]
                        )
                    # V natural for this s-chunk
                    vp = ps_q1.tile([128, HPC * DH], f32, tag="vp")
                    for jj in range(HJ):
                        nc.tensor.matmul(
                            vp,
                            lhsT=vT[:, jj, :],
                            rhs=wv_sb[:, jj, :],
                            start=(jj == 0),
                            stop=(jj == HJ - 1),
                        )
                    nc.vector.tensor_copy(v_sb[:, i, :], vp)

            # ---------------- Phase 2: attention ----------------
            with (
                tc.tile_pool(name="spat", bufs=4) as spool,
                tc.tile_pool(name="work", bufs=3) as work,
                tc.tile_pool(name="uts", bufs=2) as utpool,
                tc.tile_pool(name="small", bufs=6) as small,
                tc.tile_pool(name="ps_s", bufs=2, space="PSUM") as ps_s,
                tc.tile_pool(name="ps_t", bufs=2, space="PSUM") as ps_t,
                tc.tile_pool(name="ps_c", bufs=2, space="PSUM") as ps_c,
                tc.tile_pool(name="ps_c2", bufs=2, space="PSUM") as ps_c2,
            ):
                for p in range(HPC):
                    hp, sub = p // 2, (p % 2) * DH
                    for qi in range(SC):
                        qrows = slice(qi * 128, (qi + 1) * 128)
                        sp = spool.tile([128, S], f32, tag="sp")
                        nc.sync.dma_start(out=sp, in_=spat_ap[p, qrows, :])
                        u = work.tile([128, S], bf16, tag="u")
                        rsp = small.tile([128, KT], f32, tag="rsp")
                        for ki in range(KT):
                            kcols = slice(ki * KW, (ki + 1) * KW)
                            scp = ps_s.tile([128, KW], f32, tag="scp")
                            nc.tensor.matmul(
                                scp,
                                lhsT=qt_sb[hp][sub : sub + DH, qrows],
                                rhs=kt_sb[hp][sub : sub + DH, kcols],
                                start=True,
                                stop=True,
                            )
                            ss = work.tile([128, KW], f32, tag="ss")
                            nc.vector.tensor_add(ss, scp, sp[:, kcols])
                            nc.scalar.activation(
                                u[:, kcols],
                                ss,
                                mybir.ActivationFunctionType.Exp,
                                accum_out=rsp[:, ki : ki + 1],
                            )
                        rs = small.tile([128, 1], f32, tag="rs")
                        nc.vector.reduce_sum(rs, rsp, axis=mybir.AxisListType.X)
                        rinv = small.tile([128, 1], f32, tag="rinv")
                        nc.vector.reciprocal(rinv, rs)

                        # transpose U 128x128 blocks -> UT strip
                        ut = utpool.tile([128, SC, 128], bf16, tag="ut")
                        for kk in range(SC):
                            utp = ps_t.tile([128, 128], f32, tag="utp")
                            nc.tensor.matmul(
                                utp,
                                lhsT=u[:, kk * 128 : (kk + 1) * 128],
                                rhs=ident_bf,
                                start=True,
                                stop=True,
                            )
                            if kk % 2 == 0:
                                nc.scalar.copy(ut[:, kk, :], utp)
                            else:
                                nc.vector.tensor_copy(ut[:, kk, :], utp)

                        # PV: ctx^T [64, 128] accumulated over k chunks
                        ctxp = ps_c.tile([64, 128], f32, tag="ctxp")
                        for kk in range(SC):
                            nc.tensor.matmul(
                                ctxp,
                                lhsT=v_sb[:, kk, p * DH : (p + 1) * DH],
                                rhs=ut[:, kk, :],
                                start=(kk == 0),
                                stop=(kk == SC - 1),
                            )
                        ctxT = small.tile([64, 128], f32, tag="ctxT")
                        nc.vector.tensor_copy(ctxT, ctxp)
                        ctx2 = ps_c2.tile([128, DH], f32, tag="ctx2")
                        nc.tensor.transpose(ctx2, ctxT, ident_f[:DH, :DH])
                        o_sb = small.tile([128, DH], f32, tag="o_sb")
                        nc.vector.tensor_scalar_mul(o_sb, in0=ctx2, scalar1=rinv)
                        nc.sync.dma_start(
                            out=out_ap[qrows, p * DH : (p + 1) * DH], in_=o_sb
                        )

    nc.finalize()
    return nc


def _get_nc():
    if "nc" not in _NC_CACHE:
        _NC_CACHE["nc"] = _build_kernel()
    return _NC_CACHE["nc"]


def kernel(
    hidden_states,
    key_states,
    value_states,
    spatial_score,
    Wq,
    bq,
    Wk,
    bk,
    Wv,
    bv,
    attention_type,
    _trace=False,
):
    hidden_states = np.asarray(hidden_states, dtype=np.float32)
    key_states = np.asarray(key_states, dtype=np.float32)
    value_states = np.asarray(value_states, dtype=np.float32)
    spatial_score = np.asarray(spatial_score, dtype=np.float32)
    Wq = np.asarray(Wq, dtype=np.float32)
    Wk = np.asarray(Wk, dtype=np.float32)
    Wv = np.asarray(Wv, dtype=np.float32)
    bq = np.asarray(bq, dtype=np.float32)
    bk = np.asarray(bk, dtype=np.float32)
    bv = np.asarray(bv, dtype=np.float32)
    assert int(np.asarray(attention_type)) == 0, "only attention_type=0 supported"

    scale = 1.0 / np.sqrt(np.float32(DH))
    nc = _get_nc()

    in_maps = []
    for c in range(N_CORES):
        b = c // (N_CORES // B)
        h0 = (c % (N_CORES // B)) * HPC
        d0 = h0 * DH
        dsl = slice(d0, d0 + HPC * DH)
        in_maps.append(
            {
                "hid": hidden_states[b],
                "key": key_states[b],
                "val": value_states[b],
                "spat": spatial_score[b, h0 : h0 + HPC],
                "wq_t": np.ascontiguousarray((Wq[dsl] * scale).T),
                "wk_t": np.ascontiguousarray(Wk[dsl].T),
                "wv_t": np.ascontiguousarray(Wv[dsl].T),
                "bq2": (bq[dsl] * scale).reshape(2, 128),
                "bk2": bk[dsl].reshape(2, 128),
            }
        )

    if _trace:
        _install_ntff_hook()
    res = run_bass_kernel_spmd(
        nc, in_maps, core_ids=list(range(N_CORES)), trace=_trace
    )

    out = np.zeros((B, S, HID), dtype=np.float32)
    for c in range(N_CORES):
        b = c // (N_CORES // B)
        h0 = (c % (N_CORES // B)) * HPC
        out[b, :, h0 * DH : h0 * DH + HPC * DH] = res.results[c]["out"]
    out += bv[None, None, :]

    if _trace:
        return out, res
    return out
